# revision 8
# baseline (speedup 1.0000x reference)
"""Adaptive-softmax (AdaSoftmaxGenerator) distributed Bass kernel for 8 trn2 cores.

Strategy: vocab-parallel. Each core owns a slice of every softmax group:
  head: 2500 of 20000 direct cols (+2 replicated cluster cols, +58 pad) = 2560
  tail1: 8500 of 68000 (+204 pad) = 8704
  tail0: 5000 of 40000 (+120 pad) = 5120
Total 16384 = 32 col-tiles of 512, ordered [head | tail1 | tail0].

v2 design (vs v1 at 558us):
  - ALL matmuls fp8 DoubleRow (head included). Weights are host-scaled by 16
    so fp8 e4m3 stays in its normal range; the exp pass un-scales via the ACT
    `scale` operand and the final output is divided by 16 on the host during
    unshard. 1024 MMs x ~259ns = ~266us PE floor.
  - NO DRAM round-trip: every plane stays resident in SBUF until its group's
    AllReduce lands, then gets one fused [128,8*512] DVE add of a broadcast
    offset plane and a single output DMA. 22 rotating plane buffers (head's 5
    are reused by tail0 after head is written out).
  - Per-group row-sums AllReduced (4KB each) from a gpsimd FIFO that carries
    nothing else, so the trigger fires the instant the last exp-accum lands.
  - Weights / bias / output use tile-major DRAM layouts so each DMA descriptor
    covers a 4-8KB contiguous run (v1's row-major output produced 1KB
    descriptors and was descriptor-rate bound).
The only non-overlapped tail is: last matmul -> exp -> AllReduce(tail0) ->
offset build -> 10 plane fixups + writes (~55us).

The 2 cluster columns are computed identically on all 8 cores inside the
head region; the AllReduce over-counts them 8x, corrected by subtracting
7*exp(c) post-reduce (bit-identical across cores, so exact).
"""

import sys
import types

sys.path.insert(0, "/opt/trn_rl_repo")

import numpy as np
import ml_dtypes

import concourse.bass as bass  # noqa: F401
import concourse.mybir as mybir
import concourse.tile as tile
from concourse import bacc
from concourse.bass_utils import run_bass_kernel_spmd
from concourse.tile_rust import add_dep_helper

F32 = mybir.dt.float32
BF16 = mybir.dt.bfloat16
FP8 = mybir.dt.float8e4
AF = mybir.ActivationFunctionType
ALU = mybir.AluOpType
AX = mybir.AxisListType

NCORES = 8
B = 1024
D = 1024
P = 128
NT = 512  # col-tile width
H_OWN, T1_OWN, T0_OWN = 2500, 8500, 5000
HEAD_COLS, T1_COLS, T0_COLS = 2560, 8704, 5120  # padded per-core regions
NCOLS = HEAD_COLS + T1_COLS + T0_COLS  # 16384
NJ = 32
NTILES = (5, 17, 10)  # head, tail1, tail0
J0 = (0, 5, 22)
CL_TILE = 4  # cluster cols 2500,2501 live in head tile 4 at offsets 452,453
CL_OFF = 2500 - 4 * NT  # 452
PAD_BIAS = -10000.0
WS = 16.0  # host-side weight/bias scale; undone by ACT scale + host divide

_cached_nc = None


def build():
    nc = bacc.Bacc(None, target_bir_lowering=False, debug=False)

    xt8_d = nc.declare_dram_parameter("xt8", [P, 8 * B], FP8, isOutput=False)
    wt8_d = nc.declare_dram_parameter("wt8", [NJ * P, 8 * NT], FP8, isOutput=False)
    bias_d = nc.declare_dram_parameter("bias", [NJ * P, NT], BF16, isOutput=False)
    out_d = nc.declare_dram_parameter("out", [NJ * P, 8 * NT], BF16, isOutput=True)

    st_in = [nc.dram_tensor(f"st_in{g}", [P, 8], F32) for g in range(3)]
    st_out = [
        nc.dram_tensor(f"st_out{g}", [P, 8], F32, addr_space="Shared")
        for g in range(3)
    ]

    xt8_r = xt8_d[:, :].rearrange("p (k b) -> p k b", k=8)

    def w_slice(j):
        return wt8_d[j * P : (j + 1) * P, :].rearrange("p (k c) -> p k c", k=8)

    def out_slice(j):
        return out_d[j * P : (j + 1) * P, :].rearrange("p (bi c) -> p bi c", bi=8)

    with tile.TileContext(nc) as tc:
        with (
            tc.tile_pool(name="xt", bufs=1) as xt_pool,
            tc.tile_pool(name="w", bufs=2) as w_pool,
            tc.tile_pool(name="bias", bufs=3) as b_pool,
            tc.tile_pool(name="ps", bufs=8, space="PSUM") as ps_pool,
            tc.tile_pool(name="planes", bufs=1) as pl_pool,
            tc.tile_pool(name="exp", bufs=1) as ex_pool,
            tc.tile_pool(name="st", bufs=1) as st_pool,
            tc.tile_pool(name="ob", bufs=1) as ob_pool,
        ):
            xt8_sb = xt_pool.tile([P, 8, B], FP8, tag="xt8", name="xt8")
            nc.sync.dma_start(out=xt8_sb[:, :, :], in_=xt8_r)

            exp_t = ex_pool.tile([P, NT], BF16, tag="exp", name="exp")
            off_bc = ob_pool.tile([P, 8, NT], BF16, tag="ob", name="ob")

            # per-group exp-sum accumulator slots + small f32 workspace
            sc = [
                st_pool.tile([P, 8 * NTILES[g]], F32, tag=f"sc{g}", name=f"sc{g}")
                for g in range(3)
            ]
            ws = st_pool.tile([P, 256], F32, tag="ws", name="ws")
            c16 = [ws[:, 0:8], ws[:, 8:16]]  # 16x cluster logits (incl. bias)
            lzh = ws[:, 16:24]
            e0 = ws[:, 24:32]
            e1 = ws[:, 32:40]
            ee = ws[:, 40:48]
            strue = ws[:, 48:56]

            def sarg(g):
                return ws[:, 56 + 8 * g : 64 + 8 * g]

            def lzg(g):
                return ws[:, 80 + 8 * g : 88 + 8 * g]

            def off16(g):
                return ws[:, 104 + 8 * g : 112 + 8 * g]

            utmp = ws[:, 128:136]

            planes = {}
            ar_insts = {}

            def ptag(g, jj):
                # head: pl0-4, tail1: pl5-21, tail0 reuses pl0-9
                if g == 0:
                    return f"pl{jj}"
                if g == 1:
                    return f"pl{5 + jj}"
                return f"pl{jj}"

            def phase1_tile(g, jj):
                j = J0[g] + jj
                w_sb = w_pool.tile([P, 8, NT], FP8, tag="w", name="w")
                nc.sync.dma_start(out=w_sb[:, :, :], in_=w_slice(j))
                b_sb = b_pool.tile([P, NT], BF16, tag="bias", name="bias")
                nc.sync.dma_start(out=b_sb[:, :], in_=bias_d[j * P : (j + 1) * P, :])
                plane = pl_pool.tile(
                    [P, 8, NT], BF16, tag=ptag(g, jj), name=f"pl{g}_{jj}"
                )
                planes[(g, jj)] = plane
                nt_g = NTILES[g]
                for bi in range(8):
                    psum = ps_pool.tile([P, NT], F32, tag="ps", name="ps")
                    for k in range(4):
                        nc.tensor.matmul(
                            psum[:, :],
                            xt8_sb[:, 2 * k : 2 * k + 2, bi * P : (bi + 1) * P],
                            w_sb[:, 2 * k : 2 * k + 2, :],
                            start=(k == 0),
                            stop=(k == 3),
                            perf_mode=mybir.MatmulPerfMode.DoubleRow,
                        )
                    # plane16 = 16*(logit + bias)
                    nc.vector.tensor_tensor(
                        plane[:, bi, :], psum[:, :], b_sb[:, :], op=ALU.add
                    )
                    slot = bi * nt_g + jj
                    nc.scalar.activation(
                        exp_t[:, :],
                        plane[:, bi, :],
                        AF.Exp,
                        scale=1.0 / WS,
                        accum_out=sc[g][:, slot : slot + 1],
                    )
                    if g == 0 and jj == CL_TILE:
                        nc.vector.tensor_copy(
                            c16[0][:, bi : bi + 1], plane[:, bi, CL_OFF : CL_OFF + 1]
                        )
                        nc.vector.tensor_copy(
                            c16[1][:, bi : bi + 1],
                            plane[:, bi, CL_OFF + 1 : CL_OFF + 2],
                        )

            def stats_ar(g):
                # whole chain on gpsimd: its FIFO carries nothing else, so the
                # trigger fires the moment the last exp-accum commits.
                nt_g = NTILES[g]
                stg = ws[:, 136 + 8 * g : 144 + 8 * g]
                nc.vector.tensor_reduce(
                    stg,
                    sc[g].rearrange("p (bi t) -> p bi t", t=nt_g),
                    axis=AX.X,
                    op=ALU.add,
                )
                nc.gpsimd.dma_start(out=st_in[g][:, :], in_=stg)
                ar_insts[g] = nc.gpsimd.collective_compute(
                    "AllReduce",
                    ALU.add,
                    replica_groups=[list(range(NCORES))],
                    ins=[st_in[g][:, :]],
                    outs=[st_out[g][:, :]],
                )

            def offsets(g):
                rb = nc.gpsimd.dma_start(out=sarg(g), in_=st_out[g][:, :])
                add_dep_helper(rb.ins, ar_insts[g].ins, reason="readback-after-ar")
                if g == 0:
                    # true head sum = AR sum - 7 * (exp(c0) + exp(c1))
                    nc.scalar.activation(e0, c16[0], AF.Exp, scale=1.0 / WS)
                    nc.scalar.activation(e1, c16[1], AF.Exp, scale=1.0 / WS)
                    nc.vector.tensor_add(ee, e0, e1)
                    nc.vector.tensor_scalar_mul(ee, ee, -7.0)
                    nc.vector.tensor_add(strue, sarg(0), ee)
                    nc.scalar.activation(lzh, strue, AF.Ln)
                    # off16 = -16*lzh
                    nc.vector.tensor_scalar_mul(off16(0), lzh, -WS)
                else:
                    nc.scalar.activation(lzg(g), sarg(g), AF.Ln)
                    # off16 = c16 - 16*(lzh + lzg)
                    nc.vector.tensor_add(utmp, lzh, lzg(g))
                    nc.vector.tensor_scalar_mul(utmp, utmp, -WS)
                    # g=1 is tail1 -> cluster col 1; g=2 is tail0 -> cluster col 0
                    nc.vector.tensor_add(off16(g), c16[2 - g], utmp)

            def build_off_bc(g):
                # broadcast off16[p, bi] -> off_bc[p, bi, 0:512] on ACT
                for bi in range(8):
                    nc.scalar.activation(
                        off_bc[:, bi, :],
                        exp_t[:, :],
                        AF.Identity,
                        bias=off16(g)[:, bi : bi + 1],
                        scale=0.0,
                    )

            def fixup_add(g, jj, eng):
                plane = planes[(g, jj)]
                eng.tensor_tensor(
                    plane[:, :, :], plane[:, :, :], off_bc[:, :, :], op=ALU.add
                )

            def fixup_add_scalar(g, jj):
                # per-bi Identity+bias adds on the ACT engine (no off_bc dep)
                plane = planes[(g, jj)]
                for bi in range(8):
                    nc.scalar.activation(
                        plane[:, bi, :],
                        plane[:, bi, :],
                        AF.Identity,
                        bias=off16(g)[:, bi : bi + 1],
                        scale=1.0,
                    )

            def fixup_write(g, jj, eng):
                j = J0[g] + jj
                eng.dma_start(out=out_slice(j), in_=planes[(g, jj)][:, :, :])

            # ---- emission schedule (stream order == per-engine issue order).
            # AR-dependent ops are emitted ~5 tiles (>42us) after each AR
            # trigger so an in-order engine FIFO never parks on the AR long
            # enough to drain the 2-tile PSUM slack and stall the PE.
            for jj in range(5):
                phase1_tile(0, jj)
            stats_ar(0)
            for jj in range(0, 5):
                phase1_tile(1, jj)
            offsets(0)
            build_off_bc(0)
            phase1_tile(1, 5)
            for jj in range(5):  # head fixups on idle gpsimd (add+write in-order)
                fixup_add(0, jj, nc.gpsimd)
                fixup_write(0, jj, nc.gpsimd)
                phase1_tile(1, 6 + jj)
            for jj in range(11, 17):
                phase1_tile(1, jj)
            stats_ar(1)
            for jj in range(0, 4):
                phase1_tile(2, jj)
            offsets(1)
            build_off_bc(1)
            phase1_tile(2, 4)
            # tail1 fixups: adds on vector; writes for reused buffers (0-4) go
            # out immediately on sync; the rest lag one batch so sync/scalar
            # never park on a pending vector add.
            done = 0
            written = 0
            for jj in range(5, 10):
                # adds for tail1 planes whose buffers tile jj reuses must be
                # emitted BEFORE the reusing tile (same-FIFO WAR ordering)
                want = min(17, (jj - 4) * 4)
                while done < want:
                    fixup_add(1, done, nc.vector)
                    if done < 5:
                        fixup_write(1, done, nc.sync)
                        written += 1
                    done += 1
                phase1_tile(2, jj)
                while written < max(0, done - 4):
                    fixup_write(1, written, nc.sync if written % 2 else nc.scalar)
                    written += 1
            while done < 17:
                fixup_add(1, done, nc.vector)
                done += 1
            stats_ar(2)
            while written < 17:
                fixup_write(1, written, nc.sync if written % 2 else nc.scalar)
                written += 1
            offsets(2)
            build_off_bc(2)
            # exposed tail0 fixup: split across vector/gpsimd/scalar
            t0_eng = [None] * 10
            for jj in range(5):
                fixup_add(2, jj, nc.vector)
                t0_eng[jj] = nc.sync
            for jj in range(5, 8):
                fixup_add(2, jj, nc.gpsimd)
                t0_eng[jj] = nc.gpsimd
            for jj in range(8, 10):
                fixup_add_scalar(2, jj)
                t0_eng[jj] = nc.scalar
            for jj in range(10):
                fixup_write(2, jj, t0_eng[jj])

    nc.compile()
    return nc


def get_nc():
    global _cached_nc
    if _cached_nc is None:
        _cached_nc = build()
    return _cached_nc


def make_in_maps(x, head_w, head_b, tail0_w, tail0_b, tail1_w, tail1_b):
    f8 = ml_dtypes.float8_e4m3fn
    x = np.asarray(x, np.float32)
    # xt8[p, k, b] = x[b, k*128+p]
    xt8 = (
        np.ascontiguousarray(x.T.reshape(8, P, B).transpose(1, 0, 2))
        .reshape(P, 8 * B)
        .astype(f8)
    )
    in_maps = []
    for c in range(NCORES):
        w_parts = [
            np.asarray(head_w[c * H_OWN : (c + 1) * H_OWN], np.float32),
            np.asarray(head_w[20000:20002], np.float32),
            np.zeros((HEAD_COLS - H_OWN - 2, D), np.float32),
            np.asarray(tail1_w[c * T1_OWN : (c + 1) * T1_OWN], np.float32),
            np.zeros((T1_COLS - T1_OWN, D), np.float32),
            np.asarray(tail0_w[c * T0_OWN : (c + 1) * T0_OWN], np.float32),
            np.zeros((T0_COLS - T0_OWN, D), np.float32),
        ]
        w = np.concatenate(w_parts, axis=0) * WS  # [NCOLS, D], 16x scaled
        # wt8[j, p, k, c] = w[j*512+c, k*128+p]
        wt8 = (
            np.ascontiguousarray(
                w.reshape(NJ, NT, 8, P).transpose(0, 3, 2, 1)
            )
            .reshape(NJ * P, 8 * NT)
            .astype(f8)
        )
        b_parts = [
            np.asarray(head_b[c * H_OWN : (c + 1) * H_OWN], np.float32),
            np.asarray(head_b[20000:20002], np.float32),
            np.full(HEAD_COLS - H_OWN - 2, PAD_BIAS, np.float32),
            np.asarray(tail1_b[c * T1_OWN : (c + 1) * T1_OWN], np.float32),
            np.full(T1_COLS - T1_OWN, PAD_BIAS, np.float32),
            np.asarray(tail0_b[c * T0_OWN : (c + 1) * T0_OWN], np.float32),
            np.full(T0_COLS - T0_OWN, PAD_BIAS, np.float32),
        ]
        bias = (np.concatenate(b_parts) * WS).astype(ml_dtypes.bfloat16)  # [NCOLS]
        # bias_d[j*P+p, c] = bias[j*512+c] broadcast over p
        bias_bc = np.ascontiguousarray(
            np.broadcast_to(bias.reshape(NJ, 1, NT), (NJ, P, NT))
        ).reshape(NJ * P, NT)
        in_maps.append({"xt8": xt8, "wt8": wt8, "bias": bias_bc})
    return in_maps


def assemble(results):
    prob = np.empty((B, 128000), np.float32)
    inv = 1.0 / WS
    for c in range(NCORES):
        o = results[c]["out"].astype(np.float32)  # [NJ*P, 8*NT]
        # logical[b, col]: b = bi*128+p, col = j*512+ct
        o = o.reshape(NJ, P, 8, NT).transpose(2, 1, 0, 3).reshape(B, NCOLS) * inv
        prob[:, c * H_OWN : (c + 1) * H_OWN] = o[:, :H_OWN]
        prob[:, 60000 + c * T1_OWN : 60000 + (c + 1) * T1_OWN] = o[
            :, HEAD_COLS : HEAD_COLS + T1_OWN
        ]
        prob[:, 20000 + c * T0_OWN : 20000 + (c + 1) * T0_OWN] = o[
            :, HEAD_COLS + T1_COLS : HEAD_COLS + T1_COLS + T0_OWN
        ]
    return prob


def kernel(x, head_w, head_b, tail0_w, tail0_b, tail1_w, tail1_b):
    in_maps = make_in_maps(x, head_w, head_b, tail0_w, tail0_b, tail1_w, tail1_b)
    nc = get_nc()
    res = run_bass_kernel_spmd(nc, in_maps, core_ids=list(range(NCORES)))
    return assemble(res.results)


def run_traced(inputs):
    """Run with NTFF profiling; returns (prob, BassKernelResults)."""
    _hooks = types.ModuleType("antenv.axon_hooks")
    _hooks._hook = None
    _hooks.set_axon_ntff_profile_hook = lambda h: setattr(_hooks, "_hook", h)
    _hooks.get_axon_ntff_profile_hook = lambda: _hooks._hook
    sys.modules["antenv.axon_hooks"] = _hooks
    import antenv

    antenv.axon_hooks = _hooks
    from trn_agent_boot.trn_boot import _ntff_profile_via_ctypes

    _hooks.set_axon_ntff_profile_hook(
        _ntff_profile_via_ctypes("/opt/axon/libaxon_pjrt.so")
    )
    from concourse import bass_utils as _bu

    _bu.upload_artifacts = lambda tmpdir: tmpdir

    in_maps = make_in_maps(**inputs)
    nc = get_nc()
    res = run_bass_kernel_spmd(
        nc, in_maps, core_ids=list(range(NCORES)), trace=True
    )
    return assemble(res.results), res
